# revision 2
# baseline (speedup 1.0000x reference)
"""Trainium2 Bass kernel v3 for nn_Net_34729105555716.

Model: embedding lookup [30000,100] -> input projection (w_ih) -> 200-step
tanh RNN (hidden 300) -> relu MLP (300->256->3) over batch 4096.

Strategy (data-parallel over batch, 512 rows/core, temporal mixed precision):
  - Phase 1 (t < K8=186): recurrent matmuls in fp8e4 DoubleRow (0.5 cyc/row)
    with h stored fp8 in R [128, 4cols, 512] (cols 0-2 = h k-tiles, col 3
    permanent zeros to complete the second DoubleRow pair). Input projection
    + bias stay fp16: the gather table carries a 1.0 bias slot at row
    position 100 and w_ih^T row 100 holds 8*(b_ih+b_hh). Weights pre-scaled
    by 8 (fp8 subnormal headroom); tanh applies scale=1/8.
  - Phase 2 (t >= K8): all-fp16 recurrence (h16 [128, 3, 512]). The RNN
    forgets phase-1 fp8 quantization noise within ~50 steps, so a short
    fp16 tail restores fp16-level accuracy (measured 6e-4 absmax-rel).
  - Two half-batch streams (256 cols) pipeline ScalarE's tanh (the
    throughput floor, ~(768+222)cyc per stream-step merged activation
    reading 3 PSUM M-tiles [128,3,256]) against the PE matmuls.
  - One SWDGE transpose-gather per step (512 tokens, fp16, prefetched 2
    steps ahead) feeds the input projection directly in [emb_dim, batch]
    layout.
  - MLP head in plain fp16 with DoubleRow-free matmuls.
"""

import sys

if "/opt/trn_rl_repo" not in sys.path:
    sys.path.insert(0, "/opt/trn_rl_repo")

import numpy as np
import ml_dtypes

F8 = ml_dtypes.float8_e4m3

SEQ = 200
BATCH = 4096
VOCAB = 30000
EMB = 100
HID = 300
FC1 = 256
N_CORES = 8
BPC = BATCH // N_CORES  # batch per core
NS = 2  # streams (half-batches) pipelining tanh against matmul
SW = BPC // NS  # stream width (256)
N_RANKS = (VOCAB + 127) // 128  # 235
WS = 8.0  # weight pre-scale (recurrence + input projection)
K8 = 186  # steps run in fp8; the rest run fp16 to wash out fp8 noise
PRE = 16  # leading steps whose xe is host-gathered, hiding the table upload

_cached = {}


def _split_multiwait(nc, mybir):
    """walrus in this container rejects >1 embedded sync wait per
    instruction (>2 for EventSemaphore); split extras onto NoOp carriers."""
    n = 0
    for f in nc.m.functions:
        for blk in f.blocks:
            if not any(
                i.sync_info is not None and len(i.sync_info.on_wait) > 1
                for i in blk.instructions
            ):
                continue
            out = []
            for inst in blk.instructions:
                si = inst.sync_info
                cap = 2 if isinstance(inst, mybir.InstEventSemaphore) else 1
                if si is not None and len(si.on_wait) > cap:
                    waits = list(si.on_wait)
                    for w in waits[:-cap]:
                        n += 1
                        carrier = mybir.InstNoOp(
                            name=f"I-waitsplit-{n}", ins=[], outs=[]
                        )
                        carrier.engine = inst.engine
                        carrier.sync_info = mybir.SyncInfo(
                            on_wait=[w], on_update=[]
                        )
                        out.append(carrier)
                    si.on_wait = waits[-cap:]
                out.append(inst)
            blk.instructions = out
    return n


def _build(seq=SEQ, k8=None, split_multiwait=True):
    import concourse.bass as bass
    import concourse.mybir as mybir
    import concourse.tile as tile
    from concourse import library_config
    from concourse.tile import add_dep_helper

    if k8 is None:
        k8 = min(K8, seq - 1) if seq > 1 else 0

    dt = mybir.dt
    f8, f16, f32, i16 = dt.float8e4, dt.float16, dt.float32, dt.int16
    Tanh = mybir.ActivationFunctionType.Tanh
    Relu = mybir.ActivationFunctionType.Relu
    DR = mybir.MatmulPerfMode.DoubleRow

    nc = bass.Bass(
        "TRN2", target_bir_lowering=False, debug=False, num_devices=N_CORES,
        dynamic_dma_scratch_size=65536,
    )
    x_idx = nc.dram_tensor(
        "x_idx", [128, seq * BPC // 16], i16, kind="ExternalInput"
    )
    tbl_d = nc.dram_tensor(
        "tbl16", [128, N_RANKS * 128], f16, kind="ExternalInput"
    )
    pre = min(PRE, seq)
    xe0_d = nc.dram_tensor("xe0", [128, pre * 512], f16, kind="ExternalInput")
    wpk8_d = nc.dram_tensor("wpk8", [128, 4 * 384], f8, kind="ExternalInput")
    wih16_d = nc.dram_tensor("wih16", [128, 384], f16, kind="ExternalInput")
    whh16_d = nc.dram_tensor(
        "whh16", [128, 3 * 384], f16, kind="ExternalInput"
    )
    fc1_d = nc.dram_tensor("fc1t", [128, 3 * 256], f16, kind="ExternalInput")
    fc2_d = nc.dram_tensor("fc2t", [128, 2 * 3], f16, kind="ExternalInput")
    fc1b_d = nc.dram_tensor("fc1b", [128, 2], f32, kind="ExternalInput")
    fc2b_d = nc.dram_tensor("fc2b", [3, 1], f32, kind="ExternalInput")
    out = nc.dram_tensor("out", [3, 2, SW], f32, kind="ExternalOutput")

    with tile.TileContext(nc) as tc:
        with (
            tc.tile_pool(name="const", bufs=1) as cpool,
            tc.tile_pool(name="rpool", bufs=2) as rpool,
            tc.tile_pool(name="xpool", bufs=6) as xpool,
            tc.tile_pool(name="hpool", bufs=2) as hpool,
            tc.tile_pool(name="psum", bufs=2, space="PSUM") as ppool,
        ):
            lib_inst = nc.gpsimd.load_library(library_config.mlp)

            wpk8 = cpool.tile([128, 4, 384], f8, tag="wpk8")
            nc.sync.dma_start(wpk8[:], wpk8_d.ap())
            wih16 = cpool.tile([128, 384], f16, tag="wih16")
            nc.sync.dma_start(wih16[:], wih16_d.ap())
            whh16 = cpool.tile([128, 3, 384], f16, tag="whh16")
            nc.sync.dma_start(whh16[:], whh16_d.ap())
            f1 = cpool.tile([128, 3, 256], f16, tag="f1")
            nc.sync.dma_start(f1[:], fc1_d.ap())
            f2 = cpool.tile([128, 2, 3], f16, tag="f2")
            nc.sync.dma_start(f2[:], fc2_d.ap())
            fc1b = cpool.tile([128, 2], f32, tag="fc1b")
            nc.sync.dma_start(fc1b[:], fc1b_d.ap())
            fc2b = cpool.tile([3, 1], f32, tag="fc2b")
            nc.sync.dma_start(fc2b[:], fc2b_d.ap())
            # preloaded xe first (small), then idx + table (large): the RNN
            # starts on xe0 while the gather table is still uploading.
            xe0 = cpool.tile([128, pre, 512], f16, tag="xe0")
            nc.sync.dma_start(xe0[:], xe0_d.ap())
            idx = cpool.tile([128, seq * BPC // 16], i16, tag="idx")
            nc.sync.dma_start(idx[:], x_idx.ap())
            tbl = cpool.tile([128, N_RANKS * 128], f16, tag="tbl")
            nc.sync.dma_start(tbl[:], tbl_d.ap())

            reg_n = nc.gpsimd.to_reg(BPC)

            def gather(t):
                xg = xpool.tile([128, 1, 512], f16, tag="xg")
                gi = nc.gpsimd.dma_gather(
                    xg[:],
                    tbl[:],
                    idx[:, t * (BPC // 16) : (t + 1) * (BPC // 16)],
                    BPC,
                    reg_n,
                    128,
                    transpose=True,
                    sbuf_tokens_per_rank=128,
                    sbuf_free_dim_per_rank=256,
                )
                add_dep_helper(
                    gi.ins, lib_inst.ins, sync=False, reason="lib first"
                )
                return xg

            R = []
            for s in range(NS):
                Rs = rpool.tile([128, 3, SW], f8, tag=f"R{s}", name=f"R{s}")
                nc.vector.memset(Rs[:], 0)
                R.append(Rs)
            H = None

            PREFETCH = 5
            xgs = {u: gather(u) for u in range(pre, min(pre + PREFETCH, seq))}

            for t in range(seq):
                u = t + PREFETCH
                if pre + PREFETCH <= u < seq:
                    xgs[u] = gather(u)
                if t < pre:
                    xg = xe0[:, t : t + 1, :]
                else:
                    xg = xgs.pop(t)
                fp8_now = t < k8
                fp8_next = (t + 1) < k8
                nxt = []
                for s in range(NS):
                    c0 = s * SW
                    ps = ppool.tile([128, 4, 256], f32, tag=f"ps{s}")
                    # xe projection first: independent of h, fills the
                    # activation-latency shadow; h matmuls close the group.
                    xe_only = not fp8_now and H is None
                    # PSUM groups are per 2KB bank: cols 0,1 share bank0,
                    # col 2 is bank1 -> start on first toucher of each bank,
                    # stop on its last.
                    for mi in range(3):
                        mo = mi * 128
                        nc.tensor.matmul(
                            ps[:, mi, :], wih16[:, mo : mo + 128],
                            xg[:, 0, c0 : c0 + SW],
                            start=(mi != 1),
                            stop=(xe_only and mi != 0),
                        )
                    if fp8_now:
                        for mi in range(3):
                            mo = mi * 128
                            nc.tensor.matmul(
                                ps[:, mi, :], wpk8[:, 0:2, mo : mo + 128],
                                R[s][:, 0:2, :],
                                start=False, stop=False, perf_mode=DR,
                            )
                        r2 = R[s][:, 2:3, :].broadcast_to([128, 2, SW])
                        for mi in range(3):
                            mo = mi * 128
                            nc.tensor.matmul(
                                ps[:, mi, :], wpk8[:, 2:4, mo : mo + 128],
                                r2,
                                start=False, stop=(mi != 0), perf_mode=DR,
                            )
                    elif H is not None:
                        for ki in range(3):
                            for mi in range(3):
                                mo = mi * 128
                                nc.tensor.matmul(
                                    ps[:, mi, :], whh16[:, ki, mo : mo + 128],
                                    H[s][:, ki, :],
                                    start=False,
                                    stop=(ki == 2 and mi != 0),
                                )

                    if fp8_next:
                        dst = rpool.tile(
                            [128, 3, SW], f8, tag=f"R{s}", name=f"Rn{s}"
                        )
                    else:
                        dst = hpool.tile(
                            [128, 3, SW], f16, tag=f"H{s}", name=f"Hn{s}"
                        )
                    nxt.append(dst)
                    nc.scalar.activation(
                        dst[:], ps[:, 0:3, :], Tanh, scale=1.0 / WS
                    )
                if fp8_next:
                    R = nxt
                else:
                    H = nxt

            # MLP head (fp16): h_final is in H.
            h1 = cpool.tile([128, 2, 512], f16, tag="h1")
            for s in range(NS):
                c0 = s * SW
                ps = ppool.tile([128, 4, 256], f32, tag=f"ps{s}")
                for mi in range(2):
                    o = ps[:, mi, :]
                    for ki in range(3):
                        nc.tensor.matmul(
                            o, f1[:, ki, mi * 128 : (mi + 1) * 128],
                            H[s][:, ki, :],
                            start=(ki == 0), stop=(ki == 2),
                        )
                    nc.scalar.activation(
                        h1[:, mi, c0 : c0 + SW], o, Relu,
                        bias=fc1b[:, mi : mi + 1],
                    )
            ps2 = ppool.tile([128, 4, 256], f32, tag="ps0")
            p2 = ps2[0:3, 0:2, :]
            nc.tensor.matmul(p2, f2[:, 0, :], h1[:, 0, :], start=True, stop=False)
            nc.tensor.matmul(p2, f2[:, 1, :], h1[:, 1, :], start=False, stop=True)
            osb = cpool.tile([3, 2, SW], f32, tag="osb")
            nc.vector.tensor_scalar_add(osb[:], p2, fc2b[:, 0:1])
            nc.sync.dma_start(out.ap(), osb[:])

    mybir.codegen_inst_isa_subclasses(nc)
    if split_multiwait:
        _split_multiwait(nc, mybir)
    return nc


def _prep_inputs(x, emb, w_ih, w_hh, b_ih, b_hh, fc1_w, fc1_b, fc2_w, fc2_b,
                 seq=SEQ):
    """Marshal the model inputs into per-core DRAM input maps."""
    x = np.asarray(x)
    assert x.shape == (seq, BATCH), x.shape

    # fp16 gather table, 128-elem rows: [emb dims 0..99, 1.0 (bias carrier),
    # zeros]; SBUF layout partition = token%128, rank stripe = token//128.
    rows = np.zeros((N_RANKS * 128, 128), np.float16)
    rows[:VOCAB, :EMB] = np.asarray(emb, np.float16)
    rows[:VOCAB, EMB] = 1.0
    tbl16 = np.ascontiguousarray(
        rows.reshape(N_RANKS, 128, 128).transpose(1, 0, 2).reshape(128, -1)
    )
    pre = min(PRE, seq)

    whhT = np.asarray(w_hh, np.float32).T  # [k=300, m=300]
    wihT = np.asarray(w_ih, np.float32).T  # [k=100, m=300]
    bias = np.asarray(b_ih, np.float32) + np.asarray(b_hh, np.float32)

    # fp8 packed recurrent weights: k-tile cols 0-2 = 8*whh^T, col 3 = zeros
    Wp = np.zeros((4, 128, 384), np.float32)
    Wp[0, :, :HID] = WS * whhT[0:128]
    Wp[1, :, :HID] = WS * whhT[128:256]
    Wp[2, 0:44, :HID] = WS * whhT[256:300]
    wpk8 = np.ascontiguousarray(
        np.asarray(Wp, F8).transpose(1, 0, 2).reshape(128, -1)
    )

    # fp16 input projection (+ bias row at k=100), 8x scaled
    Wi = np.zeros((128, 384), np.float16)
    Wi[0:EMB, :HID] = np.float16(WS) * wihT.astype(np.float16)
    Wi[EMB, :HID] = (WS * bias).astype(np.float16)
    wih16 = np.ascontiguousarray(Wi)

    # fp16 recurrent weights (tail phase), 8x scaled
    Wh = np.zeros((3, 128, 384), np.float32)
    Wh[0, :, :HID] = WS * whhT[0:128]
    Wh[1, :, :HID] = WS * whhT[128:256]
    Wh[2, 0:44, :HID] = WS * whhT[256:300]
    whh16 = np.ascontiguousarray(
        Wh.astype(np.float16).transpose(1, 0, 2).reshape(128, -1)
    )

    f1T = np.asarray(fc1_w, np.float32).T  # [300, 256]
    F1 = np.zeros((3, 128, 256), np.float32)
    F1[0] = f1T[0:128]
    F1[1] = f1T[128:256]
    F1[2, 0:44] = f1T[256:300]
    fc1t = np.ascontiguousarray(
        F1.astype(np.float16).transpose(1, 0, 2).reshape(128, -1)
    )

    f2T = np.asarray(fc2_w, np.float32).T  # [256, 3]
    F2 = np.zeros((2, 128, 3), np.float32)
    F2[0] = f2T[0:128]
    F2[1] = f2T[128:256]
    fc2t = np.ascontiguousarray(
        F2.astype(np.float16).transpose(1, 0, 2).reshape(128, -1)
    )

    fc1b_sb = np.ascontiguousarray(
        np.asarray(fc1_b, np.float32).reshape(2, 128).T
    )
    fc2b_sb = np.asarray(fc2_b, np.float32).reshape(3, 1)

    shared = {
        "tbl16": tbl16,
        "wpk8": wpk8,
        "wih16": wih16,
        "whh16": whh16,
        "fc1t": fc1t,
        "fc2t": fc2t,
        "fc1b": fc1b_sb,
        "fc2b": fc2b_sb,
    }
    in_maps = []
    for c in range(N_CORES):
        xc = x[:, c * BPC : (c + 1) * BPC]  # [seq, 512]
        flat = np.ascontiguousarray(xc).reshape(-1).astype(np.int16)
        block = np.ascontiguousarray(flat.reshape(-1, 16).T)  # [16, seq*BPC/16]
        x_idx = np.ascontiguousarray(np.tile(block, (8, 1)))  # [128, ...]
        # host-gathered xe for the first `pre` steps: [128 dims, pre, 512]
        xe0 = np.ascontiguousarray(
            rows[xc[:pre]].transpose(2, 0, 1).reshape(128, -1)
        )
        in_maps.append({"x_idx": x_idx, "xe0": xe0, **shared})
    return in_maps


def _get_nc():
    if "nc" not in _cached:
        _cached["nc"] = _build()
    return _cached["nc"]


def kernel(x, emb, w_ih, w_hh, b_ih, b_hh, fc1_w, fc1_b, fc2_w, fc2_b):
    from concourse.bass_utils import run_bass_kernel_spmd

    nc = _get_nc()
    in_maps = _prep_inputs(
        x, emb, w_ih, w_hh, b_ih, b_hh, fc1_w, fc1_b, fc2_w, fc2_b
    )
    res = run_bass_kernel_spmd(nc, in_maps, core_ids=list(range(N_CORES)))
    # per-core out is [3, 2, 256] = [3, 512]; assemble full [4096, 3]
    full = np.concatenate(
        [r["out"].reshape(3, BPC).T for r in res.results], axis=0
    )
    return full.astype(np.float32)


# revision 3
# speedup vs baseline: 1.0101x; 1.0101x over previous
"""Trainium2 Bass kernel v3 for nn_Net_34729105555716.

Model: embedding lookup [30000,100] -> input projection (w_ih) -> 200-step
tanh RNN (hidden 300) -> relu MLP (300->256->3) over batch 4096.

Strategy (data-parallel over batch, 512 rows/core, temporal mixed precision):
  - Phase 1 (t < K8=186): recurrent matmuls in fp8e4 DoubleRow (0.5 cyc/row)
    with h stored fp8 in R [128, 4cols, 512] (cols 0-2 = h k-tiles, col 3
    permanent zeros to complete the second DoubleRow pair). Input projection
    + bias stay fp16: the gather table carries a 1.0 bias slot at row
    position 100 and w_ih^T row 100 holds 8*(b_ih+b_hh). Weights pre-scaled
    by 8 (fp8 subnormal headroom); tanh applies scale=1/8.
  - Phase 2 (t >= K8): all-fp16 recurrence (h16 [128, 3, 512]). The RNN
    forgets phase-1 fp8 quantization noise within ~50 steps, so a short
    fp16 tail restores fp16-level accuracy (measured 6e-4 absmax-rel).
  - Two half-batch streams (256 cols) pipeline ScalarE's tanh (the
    throughput floor, ~(768+222)cyc per stream-step merged activation
    reading 3 PSUM M-tiles [128,3,256]) against the PE matmuls.
  - One SWDGE transpose-gather per step (512 tokens, fp16, prefetched 2
    steps ahead) feeds the input projection directly in [emb_dim, batch]
    layout.
  - MLP head in plain fp16 with DoubleRow-free matmuls.
"""

import sys

if "/opt/trn_rl_repo" not in sys.path:
    sys.path.insert(0, "/opt/trn_rl_repo")

import numpy as np
import ml_dtypes

F8 = ml_dtypes.float8_e4m3

SEQ = 200
BATCH = 4096
VOCAB = 30000
EMB = 100
HID = 300
FC1 = 256
N_CORES = 8
BPC = BATCH // N_CORES  # batch per core
NS = 2  # streams (half-batches) pipelining tanh against matmul
SW = BPC // NS  # stream width (256)
N_RANKS = (VOCAB + 127) // 128  # 235
WS = 8.0  # weight pre-scale (recurrence + input projection)
K8 = 190  # steps run in fp8; the rest run fp16 to wash out fp8 noise
PRE = 16  # leading steps whose xe is host-gathered, hiding the table upload

_cached = {}


def _split_multiwait(nc, mybir):
    """walrus in this container rejects >1 embedded sync wait per
    instruction (>2 for EventSemaphore); split extras onto NoOp carriers."""
    n = 0
    for f in nc.m.functions:
        for blk in f.blocks:
            if not any(
                i.sync_info is not None and len(i.sync_info.on_wait) > 1
                for i in blk.instructions
            ):
                continue
            out = []
            for inst in blk.instructions:
                si = inst.sync_info
                cap = 2 if isinstance(inst, mybir.InstEventSemaphore) else 1
                if si is not None and len(si.on_wait) > cap:
                    waits = list(si.on_wait)
                    for w in waits[:-cap]:
                        n += 1
                        carrier = mybir.InstNoOp(
                            name=f"I-waitsplit-{n}", ins=[], outs=[]
                        )
                        carrier.engine = inst.engine
                        carrier.sync_info = mybir.SyncInfo(
                            on_wait=[w], on_update=[]
                        )
                        out.append(carrier)
                    si.on_wait = waits[-cap:]
                out.append(inst)
            blk.instructions = out
    return n


def _build(seq=SEQ, k8=None, split_multiwait=True):
    import concourse.bass as bass
    import concourse.mybir as mybir
    import concourse.tile as tile
    from concourse import library_config
    from concourse.tile import add_dep_helper

    if k8 is None:
        k8 = min(K8, seq - 1) if seq > 1 else 0

    dt = mybir.dt
    f8, f16, f32, i16 = dt.float8e4, dt.float16, dt.float32, dt.int16
    Tanh = mybir.ActivationFunctionType.Tanh
    Relu = mybir.ActivationFunctionType.Relu
    DR = mybir.MatmulPerfMode.DoubleRow

    nc = bass.Bass(
        "TRN2", target_bir_lowering=False, debug=False, num_devices=N_CORES,
        dynamic_dma_scratch_size=65536,
    )
    x_idx = nc.dram_tensor(
        "x_idx", [128, seq * BPC // 16], i16, kind="ExternalInput"
    )
    tbl_d = nc.dram_tensor(
        "tbl16", [128, N_RANKS * 128], f16, kind="ExternalInput"
    )
    pre = min(PRE, seq)
    xe0_d = nc.dram_tensor("xe0", [128, pre * 512], f16, kind="ExternalInput")
    wpk8_d = nc.dram_tensor("wpk8", [128, 4 * 384], f8, kind="ExternalInput")
    wih16_d = nc.dram_tensor("wih16", [128, 384], f16, kind="ExternalInput")
    whh16_d = nc.dram_tensor(
        "whh16", [128, 3 * 384], f16, kind="ExternalInput"
    )
    fc1_d = nc.dram_tensor("fc1t", [128, 3 * 256], f16, kind="ExternalInput")
    fc2_d = nc.dram_tensor("fc2t", [128, 2 * 3], f16, kind="ExternalInput")
    fc1b_d = nc.dram_tensor("fc1b", [128, 2], f32, kind="ExternalInput")
    fc2b_d = nc.dram_tensor("fc2b", [3, 1], f32, kind="ExternalInput")
    out = nc.dram_tensor("out", [3, 2, SW], f32, kind="ExternalOutput")

    with tile.TileContext(nc) as tc:
        with (
            tc.tile_pool(name="const", bufs=1) as cpool,
            tc.tile_pool(name="rpool", bufs=2) as rpool,
            tc.tile_pool(name="xpool", bufs=6) as xpool,
            tc.tile_pool(name="hpool", bufs=2) as hpool,
            tc.tile_pool(name="psum", bufs=2, space="PSUM") as ppool,
        ):
            lib_inst = nc.gpsimd.load_library(library_config.mlp)

            wpk8 = cpool.tile([128, 4, 384], f8, tag="wpk8")
            nc.sync.dma_start(wpk8[:], wpk8_d.ap())
            wih16 = cpool.tile([128, 384], f16, tag="wih16")
            nc.sync.dma_start(wih16[:], wih16_d.ap())
            whh16 = cpool.tile([128, 3, 384], f16, tag="whh16")
            nc.sync.dma_start(whh16[:], whh16_d.ap())
            f1 = cpool.tile([128, 3, 256], f16, tag="f1")
            nc.sync.dma_start(f1[:], fc1_d.ap())
            f2 = cpool.tile([128, 2, 3], f16, tag="f2")
            nc.sync.dma_start(f2[:], fc2_d.ap())
            fc1b = cpool.tile([128, 2], f32, tag="fc1b")
            nc.sync.dma_start(fc1b[:], fc1b_d.ap())
            fc2b = cpool.tile([3, 1], f32, tag="fc2b")
            nc.sync.dma_start(fc2b[:], fc2b_d.ap())
            # preloaded xe first (small), then idx + table (large): the RNN
            # starts on xe0 while the gather table is still uploading.
            xe0 = cpool.tile([128, pre, 512], f16, tag="xe0")
            nc.sync.dma_start(xe0[:], xe0_d.ap())
            idx = cpool.tile([128, seq * BPC // 16], i16, tag="idx")
            nc.sync.dma_start(idx[:], x_idx.ap())
            tbl = cpool.tile([128, N_RANKS * 128], f16, tag="tbl")
            nc.sync.dma_start(tbl[:], tbl_d.ap())

            reg_n = nc.gpsimd.to_reg(BPC)

            def gather(t):
                xg = xpool.tile([128, 1, 512], f16, tag="xg")
                gi = nc.gpsimd.dma_gather(
                    xg[:],
                    tbl[:],
                    idx[:, t * (BPC // 16) : (t + 1) * (BPC // 16)],
                    BPC,
                    reg_n,
                    128,
                    transpose=True,
                    sbuf_tokens_per_rank=128,
                    sbuf_free_dim_per_rank=256,
                )
                add_dep_helper(
                    gi.ins, lib_inst.ins, sync=False, reason="lib first"
                )
                return xg

            R = []
            for s in range(NS):
                Rs = rpool.tile([128, 3, SW], f8, tag=f"R{s}", name=f"R{s}")
                nc.vector.memset(Rs[:], 0)
                R.append(Rs)
            H = None

            PREFETCH = 5
            xgs = {u: gather(u) for u in range(pre, min(pre + PREFETCH, seq))}

            for t in range(seq):
                u = t + PREFETCH
                if pre + PREFETCH <= u < seq:
                    xgs[u] = gather(u)
                if t < pre:
                    xg = xe0[:, t : t + 1, :]
                else:
                    xg = xgs.pop(t)
                fp8_now = t < k8
                fp8_next = (t + 1) < k8
                nxt = []
                for s in range(NS):
                    c0 = s * SW
                    ps = ppool.tile([128, 4, 256], f32, tag=f"ps{s}")
                    # xe projection first: independent of h, fills the
                    # activation-latency shadow; h matmuls close the group.
                    xe_only = not fp8_now and H is None
                    # PSUM groups are per 2KB bank: cols 0,1 share bank0,
                    # col 2 is bank1 -> start on first toucher of each bank,
                    # stop on its last.
                    for mi in range(3):
                        mo = mi * 128
                        nc.tensor.matmul(
                            ps[:, mi, :], wih16[:, mo : mo + 128],
                            xg[:, 0, c0 : c0 + SW],
                            start=(mi != 1),
                            stop=(xe_only and mi != 0),
                        )
                    if fp8_now:
                        for mi in range(3):
                            mo = mi * 128
                            nc.tensor.matmul(
                                ps[:, mi, :], wpk8[:, 0:2, mo : mo + 128],
                                R[s][:, 0:2, :],
                                start=False, stop=False, perf_mode=DR,
                            )
                        r2 = R[s][:, 2:3, :].broadcast_to([128, 2, SW])
                        for mi in range(3):
                            mo = mi * 128
                            nc.tensor.matmul(
                                ps[:, mi, :], wpk8[:, 2:4, mo : mo + 128],
                                r2,
                                start=False, stop=(mi != 0), perf_mode=DR,
                            )
                    elif H is not None:
                        for ki in range(3):
                            for mi in range(3):
                                mo = mi * 128
                                nc.tensor.matmul(
                                    ps[:, mi, :], whh16[:, ki, mo : mo + 128],
                                    H[s][:, ki, :],
                                    start=False,
                                    stop=(ki == 2 and mi != 0),
                                )

                    if fp8_next:
                        dst = rpool.tile(
                            [128, 3, SW], f8, tag=f"R{s}", name=f"Rn{s}"
                        )
                    else:
                        dst = hpool.tile(
                            [128, 3, SW], f16, tag=f"H{s}", name=f"Hn{s}"
                        )
                    nxt.append(dst)
                    nc.scalar.activation(
                        dst[:], ps[:, 0:3, :], Tanh, scale=1.0 / WS
                    )
                if fp8_next:
                    R = nxt
                else:
                    H = nxt

            # MLP head (fp16): h_final is in H.
            h1 = cpool.tile([128, 2, 512], f16, tag="h1")
            for s in range(NS):
                c0 = s * SW
                ps = ppool.tile([128, 4, 256], f32, tag=f"ps{s}")
                for mi in range(2):
                    o = ps[:, mi, :]
                    for ki in range(3):
                        nc.tensor.matmul(
                            o, f1[:, ki, mi * 128 : (mi + 1) * 128],
                            H[s][:, ki, :],
                            start=(ki == 0), stop=(ki == 2),
                        )
                    nc.scalar.activation(
                        h1[:, mi, c0 : c0 + SW], o, Relu,
                        bias=fc1b[:, mi : mi + 1],
                    )
            ps2 = ppool.tile([128, 4, 256], f32, tag="ps0")
            p2 = ps2[0:3, 0:2, :]
            nc.tensor.matmul(p2, f2[:, 0, :], h1[:, 0, :], start=True, stop=False)
            nc.tensor.matmul(p2, f2[:, 1, :], h1[:, 1, :], start=False, stop=True)
            osb = cpool.tile([3, 2, SW], f32, tag="osb")
            nc.vector.tensor_scalar_add(osb[:], p2, fc2b[:, 0:1])
            nc.sync.dma_start(out.ap(), osb[:])

    mybir.codegen_inst_isa_subclasses(nc)
    if split_multiwait:
        _split_multiwait(nc, mybir)
    return nc


def _prep_inputs(x, emb, w_ih, w_hh, b_ih, b_hh, fc1_w, fc1_b, fc2_w, fc2_b,
                 seq=SEQ):
    """Marshal the model inputs into per-core DRAM input maps."""
    x = np.asarray(x)
    assert x.shape == (seq, BATCH), x.shape

    # fp16 gather table, 128-elem rows: [emb dims 0..99, 1.0 (bias carrier),
    # zeros]; SBUF layout partition = token%128, rank stripe = token//128.
    rows = np.zeros((N_RANKS * 128, 128), np.float16)
    rows[:VOCAB, :EMB] = np.asarray(emb, np.float16)
    rows[:VOCAB, EMB] = 1.0
    tbl16 = np.ascontiguousarray(
        rows.reshape(N_RANKS, 128, 128).transpose(1, 0, 2).reshape(128, -1)
    )
    pre = min(PRE, seq)

    whhT = np.asarray(w_hh, np.float32).T  # [k=300, m=300]
    wihT = np.asarray(w_ih, np.float32).T  # [k=100, m=300]
    bias = np.asarray(b_ih, np.float32) + np.asarray(b_hh, np.float32)

    # fp8 packed recurrent weights: k-tile cols 0-2 = 8*whh^T, col 3 = zeros
    Wp = np.zeros((4, 128, 384), np.float32)
    Wp[0, :, :HID] = WS * whhT[0:128]
    Wp[1, :, :HID] = WS * whhT[128:256]
    Wp[2, 0:44, :HID] = WS * whhT[256:300]
    wpk8 = np.ascontiguousarray(
        np.asarray(Wp, F8).transpose(1, 0, 2).reshape(128, -1)
    )

    # fp16 input projection (+ bias row at k=100), 8x scaled
    Wi = np.zeros((128, 384), np.float16)
    Wi[0:EMB, :HID] = np.float16(WS) * wihT.astype(np.float16)
    Wi[EMB, :HID] = (WS * bias).astype(np.float16)
    wih16 = np.ascontiguousarray(Wi)

    # fp16 recurrent weights (tail phase), 8x scaled
    Wh = np.zeros((3, 128, 384), np.float32)
    Wh[0, :, :HID] = WS * whhT[0:128]
    Wh[1, :, :HID] = WS * whhT[128:256]
    Wh[2, 0:44, :HID] = WS * whhT[256:300]
    whh16 = np.ascontiguousarray(
        Wh.astype(np.float16).transpose(1, 0, 2).reshape(128, -1)
    )

    f1T = np.asarray(fc1_w, np.float32).T  # [300, 256]
    F1 = np.zeros((3, 128, 256), np.float32)
    F1[0] = f1T[0:128]
    F1[1] = f1T[128:256]
    F1[2, 0:44] = f1T[256:300]
    fc1t = np.ascontiguousarray(
        F1.astype(np.float16).transpose(1, 0, 2).reshape(128, -1)
    )

    f2T = np.asarray(fc2_w, np.float32).T  # [256, 3]
    F2 = np.zeros((2, 128, 3), np.float32)
    F2[0] = f2T[0:128]
    F2[1] = f2T[128:256]
    fc2t = np.ascontiguousarray(
        F2.astype(np.float16).transpose(1, 0, 2).reshape(128, -1)
    )

    fc1b_sb = np.ascontiguousarray(
        np.asarray(fc1_b, np.float32).reshape(2, 128).T
    )
    fc2b_sb = np.asarray(fc2_b, np.float32).reshape(3, 1)

    shared = {
        "tbl16": tbl16,
        "wpk8": wpk8,
        "wih16": wih16,
        "whh16": whh16,
        "fc1t": fc1t,
        "fc2t": fc2t,
        "fc1b": fc1b_sb,
        "fc2b": fc2b_sb,
    }
    in_maps = []
    for c in range(N_CORES):
        xc = x[:, c * BPC : (c + 1) * BPC]  # [seq, 512]
        flat = np.ascontiguousarray(xc).reshape(-1).astype(np.int16)
        block = np.ascontiguousarray(flat.reshape(-1, 16).T)  # [16, seq*BPC/16]
        x_idx = np.ascontiguousarray(np.tile(block, (8, 1)))  # [128, ...]
        # host-gathered xe for the first `pre` steps: [128 dims, pre, 512]
        xe0 = np.ascontiguousarray(
            rows[xc[:pre]].transpose(2, 0, 1).reshape(128, -1)
        )
        in_maps.append({"x_idx": x_idx, "xe0": xe0, **shared})
    return in_maps


def _get_nc():
    if "nc" not in _cached:
        _cached["nc"] = _build()
    return _cached["nc"]


def kernel(x, emb, w_ih, w_hh, b_ih, b_hh, fc1_w, fc1_b, fc2_w, fc2_b):
    from concourse.bass_utils import run_bass_kernel_spmd

    nc = _get_nc()
    in_maps = _prep_inputs(
        x, emb, w_ih, w_hh, b_ih, b_hh, fc1_w, fc1_b, fc2_w, fc2_b
    )
    res = run_bass_kernel_spmd(nc, in_maps, core_ids=list(range(N_CORES)))
    # per-core out is [3, 2, 256] = [3, 512]; assemble full [4096, 3]
    full = np.concatenate(
        [r["out"].reshape(3, BPC).T for r in res.results], axis=0
    )
    return full.astype(np.float32)
